# revision 35
# baseline (speedup 1.0000x reference)
"""Causal self-attention on 8 trn2 NeuronCores.

Sharding: core = (batch b, head-group g) with b in 0..3, g in 0..1.
Each core computes, for its batch and its 8 heads (512 of 1024 embed dims):
  QT/KT projections stored transposed [e', s] (e' on partitions)
  V stored [s, e'] with a ones-column appended per head
  S^T[k, q] = K_h Q_h^T      (scores transposed; k on partitions)
  P^T = exp(S^T / 8)         (no max-subtraction; scores are O(1))
  causal zeroing of P^T via gpsimd affine_select on diagonal tiles
  att'^T[d, q] = sum_k V'_h[k, d] P^T[k, q]   (row 64 = softmax denom l)
  att_n^T = att'^T[0:64] * (1/l)  (gpsimd partition_broadcast of 1/l)
  out_partial = att_n^T.T @ Wo[rows_g, :]
Host sums the two g-partials per batch.

All matmuls run in float32r (1 cycle/row at N>=256; ~1.5e-4 norm rel err).
Head PAIRS are processed together: the two heads of an e'-tile live at
partition offsets 0/64, so their K=64 score matmuls occupy disjoint PE
row-groups and run concurrently.  Q/K projections for pair c+1 are
emitted between attention blocks of pair c to keep the PE's HAM activity
window busy (a cold PE runs at 1.2 GHz instead of 2.4).
"""
import sys

if "/opt/trn_rl_repo" not in sys.path:
    sys.path.insert(0, "/opt/trn_rl_repo")

import numpy as np

import concourse.bacc as bacc
import concourse.mybir as mybir
import concourse.tile as tile
from concourse.bass_utils import run_bass_kernel_spmd

S = 2048          # sequence length
E = 1024          # embed dim
G = 512           # per-core head-group width (8 heads x 64)
HD = 64           # head dim
NH = 8            # heads per core
EC = E // 128     # 8 E-chunks
ST = S // 128     # 16 s-tiles
SB = S // 512     # 4 s-blocks
F32 = mybir.dt.float32
F32R = mybir.dt.float32r
EXP = mybir.ActivationFunctionType.Exp
GE = mybir.AluOpType.is_ge

_CACHE = {}


def _emit(nc, tc):
    xT = nc.declare_dram_parameter("xT", [E, S], F32R, isOutput=False)
    wq = nc.declare_dram_parameter("wq", [E, G], F32R, isOutput=False)
    wk = nc.declare_dram_parameter("wk", [E, G], F32R, isOutput=False)
    wv = nc.declare_dram_parameter("wv", [E, G], F32R, isOutput=False)
    wo = nc.declare_dram_parameter("wo", [G, E], F32R, isOutput=False)
    c_ones = nc.declare_dram_parameter("c_ones", [128, 128], F32R,
                                       isOutput=False)
    out = nc.declare_dram_parameter("out", [S, E], F32, isOutput=True)

    # ---- long-lived SBUF state ----
    persist1 = tc.alloc_tile_pool(name="persist1", bufs=1, side="right")
    qT_sb, kT_sb = [], []
    for c in range(4):
        qT_sb.append(persist1.tile([128, S], F32R, name=f"qT{c}", tag=f"qT{c}"))
        kT_sb.append(persist1.tile([128, S], F32R, name=f"kT{c}", tag=f"kT{c}"))
    vP = []  # 16 x [128, 8, 65] f32r  (s on partitions; per-head V | ones)
    for st in range(ST):
        vP.append(persist1.tile([128, NH, HD + 1], F32R, name=f"vP{st}",
                                tag=f"vP{st}"))
    att_n = []  # 4 x [128, 2048] f32r (normalized attended, e' on partitions)
    for c in range(4):
        att_n.append(persist1.tile([128, S], F32R, name=f"attn{c}",
                                   tag=f"attn{c}"))
    ones_sb = persist1.tile([128, NH], F32R, name="ones_sb", tag="ones_sb")
    nc.sync.dma_start(out=ones_sb, in_=c_ones[:, 0:NH])

    # attention-phase pools allocated up front (LIFO discipline: the proj
    # pools below are released mid-kernel while these stay live)
    pst = tc.alloc_tile_pool(name="pst", bufs=2, space="PSUM")
    psatt = tc.alloc_tile_pool(name="psatt", bufs=4, space="PSUM")
    ptp = tc.alloc_tile_pool(name="ptp", bufs=8)
    smalls = tc.alloc_tile_pool(name="smalls", bufs=1)

    xpool = tc.alloc_tile_pool(name="xpool", bufs=2)
    wqk_pool = tc.alloc_tile_pool(name="wqk", bufs=1)
    pp = tc.alloc_tile_pool(name="pp", bufs=2, space="PSUM")

    def load_xtc(sb_i):
        xtc = []
        for ec in range(EC):
            t = xpool.tile([128, 512], F32R, name=f"xtc{ec}", tag=f"xtc{ec}")
            nc.sync.dma_start(
                out=t,
                in_=xT[ec * 128:(ec + 1) * 128, sb_i * 512:(sb_i + 1) * 512])
            xtc.append(t)
        return xtc

    def load_wqk(c):
        wt = {}
        for wname, wdram in (("q", wq), ("k", wk)):
            for ec in range(EC):
                t = wqk_pool.tile([128, 128], F32R, name=f"w{wname}{ec}",
                                  tag=f"w{wname}{ec}")
                nc.sync.dma_start(
                    out=t,
                    in_=wdram[ec * 128:(ec + 1) * 128, c * 128:(c + 1) * 128])
                wt[(wname, ec)] = t
        return wt

    def qk_proj(c, sb_i, xtc, wt):
        for wname, dest in (("q", qT_sb), ("k", kT_sb)):
            ps = pp.tile([128, 512], F32, name="ps_proj", tag="ps_proj")
            for ec in range(EC):
                nc.tensor.matmul(ps, lhsT=wt[(wname, ec)], rhs=xtc[ec],
                                 start=(ec == 0), stop=(ec == EC - 1),
                                 skip_group_check=True)
            nc.vector.tensor_copy(
                dest[c][:, sb_i * 512:(sb_i + 1) * 512], ps)

    # ---- pass A: pair-0 Q/K projections + all V projections ----
    wv_pool = tc.alloc_tile_pool(name="wvpool", bufs=1)
    wt0 = load_wqk(0)
    xtc0 = load_xtc(0)
    wv_t = []
    for ec in range(EC):
        t = wv_pool.tile([128, G], F32R, name=f"wv{ec}", tag=f"wv{ec}")
        nc.sync.dma_start(out=t, in_=wv[ec * 128:(ec + 1) * 128, :])
        wv_t.append(t)
    for sb_i in range(SB):
        xtc = xtc0 if sb_i == 0 else load_xtc(sb_i)
        qk_proj(0, sb_i, xtc, wt0)
        for s4 in range(4):
            st = sb_i * 4 + s4
            ps = pp.tile([128, 512], F32, name="ps_proj", tag="ps_proj")
            for ec in range(EC):
                nc.tensor.matmul(ps, lhsT=xtc[ec][:, s4 * 128:(s4 + 1) * 128],
                                 rhs=wv_t[ec],
                                 start=(ec == 0), stop=(ec == EC - 1),
                                 skip_group_check=True)
            nc.vector.tensor_copy(vP[st][:, :, 0:HD],
                                  ps.rearrange("p (h d) -> p h d", h=NH))
            # softmax-denominator ones column (col 64 of each head)
            nc.vector.tensor_copy(vP[st][:, :, HD], ones_sb)
    wv_pool.release()

    def attention_block(c, qb, apool=None):
        apool = apool or psatt
        last_kt = 4 * qb + 3
        att_ps = [apool.tile([HD + 1, 512], F32, name="att_ps",
                             tag="att_ps") for _ in range(2)]
        for kt in range(last_kt + 1):
            if kt < 4 * qb:
                cs, diag = 0, False
            else:
                d0 = 128 * kt - 512 * qb
                cs, diag = min(d0, 256), True
            w = 512 - cs
            for u in range(2):
                po = u * HD
                h = 2 * c + u
                s_ps = pst.tile([128, 512], F32, name="s_ps", tag="s_ps")
                nc.tensor.matmul(
                    s_ps[:, cs:512],
                    lhsT=kT_sb[c][po:po + HD, kt * 128:(kt + 1) * 128],
                    rhs=qT_sb[c][po:po + HD, qb * 512 + cs:(qb + 1) * 512],
                    start=True, stop=True, skip_group_check=True,
                    tile_position=(po, 0))
                pt = ptp.tile([128, 512], F32R, name="pt", tag="pt")
                nc.scalar.activation(
                    pt[:, cs:512], s_ps[:, cs:512], EXP, scale=0.125)
                if diag:
                    # zero invalid (k > q):
                    # valid iff (512*qb + cs + y) - (128*kt + x) >= 0
                    nc.gpsimd.affine_select(
                        out=pt[:, cs:512], in_=pt[:, cs:512],
                        compare_op=GE, fill=0.0,
                        base=512 * qb + cs - 128 * kt,
                        channel_multiplier=-1,
                        pattern=[[1, w]])
                nc.tensor.matmul(
                    att_ps[u][:, cs:512],
                    lhsT=vP[kt][:, h, :],
                    rhs=pt[:, cs:512],
                    start=(kt == 0), stop=(kt == last_kt),
                    skip_group_check=True)
        for u in range(2):
            po = u * HD
            l_sb = smalls.tile([1, 512], F32, name="l_sb", tag="l_sb")
            nc.vector.tensor_copy(l_sb, att_ps[u][HD:HD + 1, :])
            r_sb = smalls.tile([1, 512], F32, name="r_sb", tag="r_sb")
            nc.vector.reciprocal_approx_fast(out=r_sb, in_=l_sb)
            rb_sb = smalls.tile([HD, 512], F32, name="rb_sb", tag="rb_sb")
            nc.gpsimd.partition_broadcast(rb_sb, r_sb)
            nc.vector.tensor_mul(
                att_n[c][po:po + HD, qb * 512:(qb + 1) * 512],
                att_ps[u][0:HD, :], rb_sb)

    def outproj(qb):
        for s4 in range(4):
            st = qb * 4 + s4
            for eb in range(2):
                ps = po_pool.tile([128, 512], F32, name="ps_o", tag="ps_o")
                for c in range(4):
                    nc.tensor.matmul(
                        ps,
                        lhsT=att_n[c][:, st * 128:(st + 1) * 128],
                        rhs=wo_sb[c][:, eb * 512:(eb + 1) * 512],
                        start=(c == 0), stop=(c == 3), skip_group_check=True)
                o_sb = ostage.tile([128, 512], F32, name="o_sb", tag="o_sb")
                nc.vector.tensor_copy(o_sb, ps)
                nc.sync.dma_start(
                    out=out[st * 128:(st + 1) * 128,
                            eb * 512:(eb + 1) * 512],
                    in_=o_sb)

    # ---- wavefront: Q/K projections for pairs 1..3 interleaved with ----
    # ---- attention blocks of already-projected pairs                ----
    for c in range(1, 4):
        wt = load_wqk(c)
        for sb_i in range(SB):
            xtc = load_xtc(sb_i)
            qk_proj(c, sb_i, xtc, wt)
        # attention anti-diagonal: blocks with pair + qb budget available
        for cc in range(c):
            qb = c - 1 - cc
            attention_block(cc, qb)
    wqk_pool.release()
    xpool.release()
    pp.release()

    po_pool = tc.alloc_tile_pool(name="po", bufs=2, space="PSUM")
    wopool = tc.alloc_tile_pool(name="wopool", bufs=1)
    ostage = tc.alloc_tile_pool(name="ostage", bufs=2)
    wo_sb = []
    for c in range(4):
        t = wopool.tile([128, E], F32R, name=f"wo{c}", tag=f"wo{c}")
        nc.sync.dma_start(out=t, in_=wo[c * 128:(c + 1) * 128, :])
        wo_sb.append(t)

    # remaining anti-diagonals; outproj(qb) as soon as all pairs reach qb
    for d in range(3, 7):
        blocks = [(cc, d - cc) for cc in range(4) if 0 <= d - cc <= 3]
        for i, (cc, qb) in enumerate(blocks):
            attention_block(cc, qb)
            if i == len(blocks) - 1:
                outproj(d - 3)

    # release in LIFO order per memory space
    ostage.release()
    wopool.release()
    smalls.release()
    ptp.release()
    po_pool.release()
    psatt.release()
    pst.release()
    persist1.release()


def _build():
    if "nc" in _CACHE:
        return _CACHE["nc"]
    nc = bacc.Bacc()
    with tile.TileContext(nc) as tc:
        _emit(nc, tc)
    nc.compile()
    _CACHE["nc"] = nc
    return nc


def _make_in_maps(inputs):
    x = np.asarray(inputs["x"], dtype=np.float32)
    Wq = np.asarray(inputs["Wq"], dtype=np.float32)
    Wk = np.asarray(inputs["Wk"], dtype=np.float32)
    Wv = np.asarray(inputs["Wv"], dtype=np.float32)
    Wo = np.asarray(inputs["Wo"], dtype=np.float32)
    in_maps = []
    for core in range(8):
        b, g = core // 2, core % 2
        cols = slice(g * G, (g + 1) * G)
        in_maps.append({
            "xT": np.ascontiguousarray(x[b].T),
            "wq": np.ascontiguousarray(Wq[:, cols]),
            "wk": np.ascontiguousarray(Wk[:, cols]),
            "wv": np.ascontiguousarray(Wv[:, cols]),
            "wo": np.ascontiguousarray(Wo[cols, :]),
            "c_ones": np.ones((128, 128), dtype=np.float32),
        })
    return in_maps


def kernel(x, Wq, Wk, Wv, Wo):
    nc = _build()
    in_maps = _make_in_maps(dict(x=x, Wq=Wq, Wk=Wk, Wv=Wv, Wo=Wo))
    res = run_bass_kernel_spmd(nc, in_maps, core_ids=list(range(8)))
    out = np.zeros((4, S, E), dtype=np.float32)
    for core in range(8):
        out[core // 2] += res.results[core]["out"]
    return out


if __name__ == "__main__":
    rng = np.random.default_rng(0)
    x = rng.standard_normal((4, S, E), dtype=np.float32)
    sc = 1.0 / np.sqrt(E)
    Wq = rng.standard_normal((E, E), dtype=np.float32) * sc
    Wk = rng.standard_normal((E, E), dtype=np.float32) * sc
    Wv = rng.standard_normal((E, E), dtype=np.float32) * sc
    Wo = rng.standard_normal((E, E), dtype=np.float32) * sc
    o = kernel(x, Wq, Wk, Wv, Wo)
    print("out", o.shape, o.dtype, np.abs(o).mean())


# revision 36
# speedup vs baseline: 1.1291x; 1.1291x over previous
"""Causal self-attention on 8 trn2 NeuronCores.

Sharding: core = (batch b, head-group g) with b in 0..3, g in 0..1.
Each core computes, for its batch and its 8 heads (512 of 1024 embed dims):
  QT/KT projections stored transposed [e', s] (e' on partitions)
  V stored [s, e'] with a ones-column appended per head
  S^T[k, q] = K_h Q_h^T      (scores transposed; k on partitions)
  P^T = exp(S^T / 8)         (no max-subtraction; scores are O(1))
  causal zeroing of P^T via gpsimd affine_select on diagonal tiles
  att'^T[d, q] = sum_k V'_h[k, d] P^T[k, q]   (row 64 = softmax denom l)
  att_n^T = att'^T[0:64] * (1/l)  (gpsimd partition_broadcast of 1/l)
  out_partial = att_n^T.T @ Wo[rows_g, :]
Host sums the two g-partials per batch.

All matmuls run in float32r (1 cycle/row at N>=256; ~1.5e-4 norm rel err).
Head PAIRS are processed together: the two heads of an e'-tile live at
partition offsets 0/64, so their K=64 score matmuls occupy disjoint PE
row-groups and run concurrently.  Q/K projections for pair c+1 are
emitted between attention blocks of pair c to keep the PE's HAM activity
window busy (a cold PE runs at 1.2 GHz instead of 2.4).
"""
import sys

if "/opt/trn_rl_repo" not in sys.path:
    sys.path.insert(0, "/opt/trn_rl_repo")

import numpy as np

import concourse.bacc as bacc
import concourse.mybir as mybir
import concourse.tile as tile
from concourse.bass_utils import run_bass_kernel_spmd

S = 2048          # sequence length
E = 1024          # embed dim
G = 512           # per-core head-group width (8 heads x 64)
HD = 64           # head dim
NH = 8            # heads per core
EC = E // 128     # 8 E-chunks
ST = S // 128     # 16 s-tiles
SB = S // 512     # 4 s-blocks
F32 = mybir.dt.float32
F32R = mybir.dt.float32r
EXP = mybir.ActivationFunctionType.Exp
GE = mybir.AluOpType.is_ge

_CACHE = {}


def _emit(nc, tc):
    xT = nc.declare_dram_parameter("xT", [E, S], F32R, isOutput=False)
    wq = nc.declare_dram_parameter("wq", [E, G], F32R, isOutput=False)
    wk = nc.declare_dram_parameter("wk", [E, G], F32R, isOutput=False)
    wv = nc.declare_dram_parameter("wv", [E, G], F32R, isOutput=False)
    wo = nc.declare_dram_parameter("wo", [G, E], F32R, isOutput=False)
    c_ones = nc.declare_dram_parameter("c_ones", [128, 128], F32R,
                                       isOutput=False)
    out = nc.declare_dram_parameter("out", [S, E], F32, isOutput=True)

    # ---- long-lived SBUF state ----
    persist1 = tc.alloc_tile_pool(name="persist1", bufs=1, side="right")
    qT_sb, kT_sb = [], []
    for c in range(4):
        qT_sb.append(persist1.tile([128, S], F32R, name=f"qT{c}", tag=f"qT{c}"))
        kT_sb.append(persist1.tile([128, S], F32R, name=f"kT{c}", tag=f"kT{c}"))
    vP = []  # 16 x [128, 8, 65] f32r  (s on partitions; per-head V | ones)
    for st in range(ST):
        vP.append(persist1.tile([128, NH, HD + 1], F32R, name=f"vP{st}",
                                tag=f"vP{st}"))
    att_n = []  # 4 x [128, 2048] f32r (normalized attended, e' on partitions)
    for c in range(4):
        att_n.append(persist1.tile([128, S], F32R, name=f"attn{c}",
                                   tag=f"attn{c}"))
    ones_sb = persist1.tile([128, NH], F32R, name="ones_sb", tag="ones_sb")
    nc.sync.dma_start(out=ones_sb, in_=c_ones[:, 0:NH])

    # attention-phase pools allocated up front (LIFO discipline: the proj
    # pools below are released mid-kernel while these stay live)
    pst = tc.alloc_tile_pool(name="pst", bufs=3, space="PSUM")
    psatt = tc.alloc_tile_pool(name="psatt", bufs=3, space="PSUM")
    ptp = tc.alloc_tile_pool(name="ptp", bufs=8)
    smalls = tc.alloc_tile_pool(name="smalls", bufs=1)

    xpool = tc.alloc_tile_pool(name="xpool", bufs=2)
    wqk_pool = tc.alloc_tile_pool(name="wqk", bufs=1)
    pp = tc.alloc_tile_pool(name="pp", bufs=2, space="PSUM")

    def load_xtc(sb_i):
        xtc = []
        for ec in range(EC):
            t = xpool.tile([128, 512], F32R, name=f"xtc{ec}", tag=f"xtc{ec}")
            nc.sync.dma_start(
                out=t,
                in_=xT[ec * 128:(ec + 1) * 128, sb_i * 512:(sb_i + 1) * 512])
            xtc.append(t)
        return xtc

    def load_wqk(c):
        wt = {}
        for wname, wdram in (("q", wq), ("k", wk)):
            for ec in range(EC):
                t = wqk_pool.tile([128, 128], F32R, name=f"w{wname}{ec}",
                                  tag=f"w{wname}{ec}")
                nc.sync.dma_start(
                    out=t,
                    in_=wdram[ec * 128:(ec + 1) * 128, c * 128:(c + 1) * 128])
                wt[(wname, ec)] = t
        return wt

    def qk_proj(c, sb_i, xtc, wt):
        for wname, dest in (("q", qT_sb), ("k", kT_sb)):
            ps = pp.tile([128, 512], F32, name="ps_proj", tag="ps_proj")
            for ec in range(EC):
                nc.tensor.matmul(ps, lhsT=wt[(wname, ec)], rhs=xtc[ec],
                                 start=(ec == 0), stop=(ec == EC - 1),
                                 skip_group_check=True)
            nc.vector.tensor_copy(
                dest[c][:, sb_i * 512:(sb_i + 1) * 512], ps)

    # ---- pass A: pair-0 Q/K projections + all V projections ----
    wv_pool = tc.alloc_tile_pool(name="wvpool", bufs=1)
    wt0 = load_wqk(0)
    xtc0 = load_xtc(0)
    wv_t = []
    for ec in range(EC):
        t = wv_pool.tile([128, G], F32R, name=f"wv{ec}", tag=f"wv{ec}")
        nc.sync.dma_start(out=t, in_=wv[ec * 128:(ec + 1) * 128, :])
        wv_t.append(t)
    for sb_i in range(SB):
        xtc = xtc0 if sb_i == 0 else load_xtc(sb_i)
        qk_proj(0, sb_i, xtc, wt0)
        for s4 in range(4):
            st = sb_i * 4 + s4
            ps = pp.tile([128, 512], F32, name="ps_proj", tag="ps_proj")
            for ec in range(EC):
                nc.tensor.matmul(ps, lhsT=xtc[ec][:, s4 * 128:(s4 + 1) * 128],
                                 rhs=wv_t[ec],
                                 start=(ec == 0), stop=(ec == EC - 1),
                                 skip_group_check=True)
            nc.vector.tensor_copy(vP[st][:, :, 0:HD],
                                  ps.rearrange("p (h d) -> p h d", h=NH))
            # softmax-denominator ones column (col 64 of each head)
            nc.vector.tensor_copy(vP[st][:, :, HD], ones_sb)
    wv_pool.release()

    def attention_block(c, qb, apool=None):
        apool = apool or psatt
        last_kt = 4 * qb + 3
        att_ps = [apool.tile([HD + 1, 512], F32, name="att_ps",
                             tag="att_ps") for _ in range(2)]
        for kt in range(last_kt + 1):
            if kt < 4 * qb:
                cs, diag = 0, False
            else:
                d0 = 128 * kt - 512 * qb
                cs, diag = min(d0, 256), True
            w = 512 - cs
            for u in range(2):
                po = u * HD
                h = 2 * c + u
                s_ps = pst.tile([128, 512], F32, name="s_ps", tag="s_ps")
                nc.tensor.matmul(
                    s_ps[:, cs:512],
                    lhsT=kT_sb[c][po:po + HD, kt * 128:(kt + 1) * 128],
                    rhs=qT_sb[c][po:po + HD, qb * 512 + cs:(qb + 1) * 512],
                    start=True, stop=True, skip_group_check=True,
                    tile_position=(po, 0))
                pt = ptp.tile([128, 512], F32R, name="pt", tag="pt")
                nc.scalar.activation(
                    pt[:, cs:512], s_ps[:, cs:512], EXP, scale=0.125)
                if diag:
                    # zero invalid (k > q):
                    # valid iff (512*qb + cs + y) - (128*kt + x) >= 0
                    nc.gpsimd.affine_select(
                        out=pt[:, cs:512], in_=pt[:, cs:512],
                        compare_op=GE, fill=0.0,
                        base=512 * qb + cs - 128 * kt,
                        channel_multiplier=-1,
                        pattern=[[1, w]])
                nc.tensor.matmul(
                    att_ps[u][:, cs:512],
                    lhsT=vP[kt][:, h, :],
                    rhs=pt[:, cs:512],
                    start=(kt == 0), stop=(kt == last_kt),
                    skip_group_check=True)
        for u in range(2):
            po = u * HD
            l_sb = smalls.tile([1, 512], F32, name="l_sb", tag="l_sb")
            nc.vector.tensor_copy(l_sb, att_ps[u][HD:HD + 1, :])
            r_sb = smalls.tile([1, 512], F32, name="r_sb", tag="r_sb")
            nc.vector.reciprocal_approx_fast(out=r_sb, in_=l_sb)
            rb_sb = smalls.tile([HD, 512], F32, name="rb_sb", tag="rb_sb")
            nc.gpsimd.partition_broadcast(rb_sb, r_sb)
            nc.vector.tensor_mul(
                att_n[c][po:po + HD, qb * 512:(qb + 1) * 512],
                att_ps[u][0:HD, :], rb_sb)

    def outproj(qb):
        for s4 in range(4):
            st = qb * 4 + s4
            for eb in range(2):
                ps = po_pool.tile([128, 512], F32, name="ps_o", tag="ps_o")
                for c in range(4):
                    nc.tensor.matmul(
                        ps,
                        lhsT=att_n[c][:, st * 128:(st + 1) * 128],
                        rhs=wo_sb[c][:, eb * 512:(eb + 1) * 512],
                        start=(c == 0), stop=(c == 3), skip_group_check=True)
                o_sb = ostage.tile([128, 512], F32, name="o_sb", tag="o_sb")
                nc.vector.tensor_copy(o_sb, ps)
                nc.sync.dma_start(
                    out=out[st * 128:(st + 1) * 128,
                            eb * 512:(eb + 1) * 512],
                    in_=o_sb)

    # ---- wavefront: Q/K projections for pairs 1..3 interleaved with ----
    # ---- attention blocks of already-projected pairs                ----
    for c in range(1, 4):
        wt = load_wqk(c)
        for sb_i in range(SB):
            xtc = load_xtc(sb_i)
            qk_proj(c, sb_i, xtc, wt)
        # attention anti-diagonal: blocks with pair + qb budget available
        for cc in range(c):
            qb = c - 1 - cc
            attention_block(cc, qb)
    wqk_pool.release()
    xpool.release()
    pp.release()

    po_pool = tc.alloc_tile_pool(name="po", bufs=2, space="PSUM")
    wopool = tc.alloc_tile_pool(name="wopool", bufs=1)
    ostage = tc.alloc_tile_pool(name="ostage", bufs=2)
    wo_sb = []
    for c in range(4):
        t = wopool.tile([128, E], F32R, name=f"wo{c}", tag=f"wo{c}")
        nc.sync.dma_start(out=t, in_=wo[c * 128:(c + 1) * 128, :])
        wo_sb.append(t)

    # remaining anti-diagonals; outproj(qb) as soon as all pairs reach qb
    for d in range(3, 7):
        blocks = [(cc, d - cc) for cc in range(4) if 0 <= d - cc <= 3]
        for i, (cc, qb) in enumerate(blocks):
            attention_block(cc, qb)
            if i == len(blocks) - 1:
                outproj(d - 3)

    # release in LIFO order per memory space
    ostage.release()
    wopool.release()
    smalls.release()
    ptp.release()
    po_pool.release()
    psatt.release()
    pst.release()
    persist1.release()


def _build():
    if "nc" in _CACHE:
        return _CACHE["nc"]
    nc = bacc.Bacc()
    with tile.TileContext(nc) as tc:
        _emit(nc, tc)
    nc.compile()
    _CACHE["nc"] = nc
    return nc


def _make_in_maps(inputs):
    x = np.asarray(inputs["x"], dtype=np.float32)
    Wq = np.asarray(inputs["Wq"], dtype=np.float32)
    Wk = np.asarray(inputs["Wk"], dtype=np.float32)
    Wv = np.asarray(inputs["Wv"], dtype=np.float32)
    Wo = np.asarray(inputs["Wo"], dtype=np.float32)
    in_maps = []
    for core in range(8):
        b, g = core // 2, core % 2
        cols = slice(g * G, (g + 1) * G)
        in_maps.append({
            "xT": np.ascontiguousarray(x[b].T),
            "wq": np.ascontiguousarray(Wq[:, cols]),
            "wk": np.ascontiguousarray(Wk[:, cols]),
            "wv": np.ascontiguousarray(Wv[:, cols]),
            "wo": np.ascontiguousarray(Wo[cols, :]),
            "c_ones": np.ones((128, 128), dtype=np.float32),
        })
    return in_maps


def kernel(x, Wq, Wk, Wv, Wo):
    nc = _build()
    in_maps = _make_in_maps(dict(x=x, Wq=Wq, Wk=Wk, Wv=Wv, Wo=Wo))
    res = run_bass_kernel_spmd(nc, in_maps, core_ids=list(range(8)))
    out = np.zeros((4, S, E), dtype=np.float32)
    for core in range(8):
        out[core // 2] += res.results[core]["out"]
    return out


if __name__ == "__main__":
    rng = np.random.default_rng(0)
    x = rng.standard_normal((4, S, E), dtype=np.float32)
    sc = 1.0 / np.sqrt(E)
    Wq = rng.standard_normal((E, E), dtype=np.float32) * sc
    Wk = rng.standard_normal((E, E), dtype=np.float32) * sc
    Wv = rng.standard_normal((E, E), dtype=np.float32) * sc
    Wo = rng.standard_normal((E, E), dtype=np.float32) * sc
    o = kernel(x, Wq, Wk, Wv, Wo)
    print("out", o.shape, o.dtype, np.abs(o).mean())


# revision 37
# speedup vs baseline: 1.2077x; 1.0696x over previous
"""Causal self-attention on 8 trn2 NeuronCores.

Sharding: core = (batch b, head-group g) with b in 0..3, g in 0..1.
Each core computes, for its batch and its 8 heads (512 of 1024 embed dims):
  QT/KT projections stored transposed [e', s] (e' on partitions)
  V stored [s, e'] with a ones-column appended per head
  S^T[k, q] = K_h Q_h^T      (scores transposed; k on partitions)
  P^T = exp(S^T / 8)         (no max-subtraction; scores are O(1))
  causal zeroing of P^T via gpsimd affine_select on diagonal tiles
  att'^T[d, q] = sum_k V'_h[k, d] P^T[k, q]   (row 64 = softmax denom l)
  att_n^T = att'^T[0:64] * (1/l)  (gpsimd partition_broadcast of 1/l)
  out_partial = att_n^T.T @ Wo[rows_g, :]
Host sums the two g-partials per batch.

All matmuls run in float32r (1 cycle/row at N>=256; ~1.5e-4 norm rel err).
Head PAIRS are processed together: the two heads of an e'-tile live at
partition offsets 0/64, so their K=64 score matmuls occupy disjoint PE
row-groups and run concurrently.  Q/K projections for pair c+1 are
emitted between attention blocks of pair c to keep the PE's HAM activity
window busy (a cold PE runs at 1.2 GHz instead of 2.4).
"""
import sys

if "/opt/trn_rl_repo" not in sys.path:
    sys.path.insert(0, "/opt/trn_rl_repo")

import numpy as np

import concourse.bacc as bacc
import concourse.mybir as mybir
import concourse.tile as tile
from concourse.bass_utils import run_bass_kernel_spmd

S = 2048          # sequence length
E = 1024          # embed dim
G = 512           # per-core head-group width (8 heads x 64)
HD = 64           # head dim
NH = 8            # heads per core
EC = E // 128     # 8 E-chunks
ST = S // 128     # 16 s-tiles
SB = S // 512     # 4 s-blocks
F32 = mybir.dt.float32
F32R = mybir.dt.float32r
EXP = mybir.ActivationFunctionType.Exp
GE = mybir.AluOpType.is_ge

_CACHE = {}


def _emit(nc, tc):
    xT = nc.declare_dram_parameter("xT", [E, S], F32R, isOutput=False)
    wq = nc.declare_dram_parameter("wq", [E, G], F32R, isOutput=False)
    wk = nc.declare_dram_parameter("wk", [E, G], F32R, isOutput=False)
    wv = nc.declare_dram_parameter("wv", [E, G], F32R, isOutput=False)
    wo = nc.declare_dram_parameter("wo", [G, E], F32R, isOutput=False)
    c_ones = nc.declare_dram_parameter("c_ones", [128, 128], F32R,
                                       isOutput=False)
    out = nc.declare_dram_parameter("out", [S, E], F32, isOutput=True)

    # ---- long-lived SBUF state ----
    persist1 = tc.alloc_tile_pool(name="persist1", bufs=1, side="right")
    qT_sb, kT_sb = [], []
    for c in range(4):
        qT_sb.append(persist1.tile([128, S], F32R, name=f"qT{c}", tag=f"qT{c}"))
        kT_sb.append(persist1.tile([128, S], F32R, name=f"kT{c}", tag=f"kT{c}"))
    vP = []  # 16 x [128, 8, 65] f32r  (s on partitions; per-head V | ones)
    for st in range(ST):
        vP.append(persist1.tile([128, NH, HD + 1], F32R, name=f"vP{st}",
                                tag=f"vP{st}"))
    att_n = []  # 4 x [128, 2048] f32r (normalized attended, e' on partitions)
    for c in range(4):
        att_n.append(persist1.tile([128, S], F32R, name=f"attn{c}",
                                   tag=f"attn{c}"))
    ones_sb = persist1.tile([128, NH], F32R, name="ones_sb", tag="ones_sb")
    nc.sync.dma_start(out=ones_sb, in_=c_ones[:, 0:NH])

    # attention-phase pools allocated up front (LIFO discipline: the proj
    # pools below are released mid-kernel while these stay live)
    pst = tc.alloc_tile_pool(name="pst", bufs=4, space="PSUM")
    psatt = tc.alloc_tile_pool(name="psatt", bufs=2, space="PSUM")
    ptp = tc.alloc_tile_pool(name="ptp", bufs=8)
    smalls = tc.alloc_tile_pool(name="smalls", bufs=1)

    xpool = tc.alloc_tile_pool(name="xpool", bufs=2)
    wqk_pool = tc.alloc_tile_pool(name="wqk", bufs=1)
    pp = tc.alloc_tile_pool(name="pp", bufs=2, space="PSUM")

    def load_xtc(sb_i):
        xtc = []
        for ec in range(EC):
            t = xpool.tile([128, 512], F32R, name=f"xtc{ec}", tag=f"xtc{ec}")
            nc.sync.dma_start(
                out=t,
                in_=xT[ec * 128:(ec + 1) * 128, sb_i * 512:(sb_i + 1) * 512])
            xtc.append(t)
        return xtc

    def load_wqk(c):
        wt = {}
        for wname, wdram in (("q", wq), ("k", wk)):
            for ec in range(EC):
                t = wqk_pool.tile([128, 128], F32R, name=f"w{wname}{ec}",
                                  tag=f"w{wname}{ec}")
                nc.sync.dma_start(
                    out=t,
                    in_=wdram[ec * 128:(ec + 1) * 128, c * 128:(c + 1) * 128])
                wt[(wname, ec)] = t
        return wt

    def qk_proj(c, sb_i, xtc, wt):
        for wname, dest in (("q", qT_sb), ("k", kT_sb)):
            ps = pp.tile([128, 512], F32, name="ps_proj", tag="ps_proj")
            for ec in range(EC):
                nc.tensor.matmul(ps, lhsT=wt[(wname, ec)], rhs=xtc[ec],
                                 start=(ec == 0), stop=(ec == EC - 1),
                                 skip_group_check=True)
            nc.vector.tensor_copy(
                dest[c][:, sb_i * 512:(sb_i + 1) * 512], ps)

    # ---- pass A: pair-0 Q/K projections + all V projections ----
    wv_pool = tc.alloc_tile_pool(name="wvpool", bufs=1)
    wt0 = load_wqk(0)
    xtc0 = load_xtc(0)
    wv_t = []
    for ec in range(EC):
        t = wv_pool.tile([128, G], F32R, name=f"wv{ec}", tag=f"wv{ec}")
        nc.sync.dma_start(out=t, in_=wv[ec * 128:(ec + 1) * 128, :])
        wv_t.append(t)
    for sb_i in range(SB):
        xtc = xtc0 if sb_i == 0 else load_xtc(sb_i)
        qk_proj(0, sb_i, xtc, wt0)
        for s4 in range(4):
            st = sb_i * 4 + s4
            ps = pp.tile([128, 512], F32, name="ps_proj", tag="ps_proj")
            for ec in range(EC):
                nc.tensor.matmul(ps, lhsT=xtc[ec][:, s4 * 128:(s4 + 1) * 128],
                                 rhs=wv_t[ec],
                                 start=(ec == 0), stop=(ec == EC - 1),
                                 skip_group_check=True)
            nc.vector.tensor_copy(vP[st][:, :, 0:HD],
                                  ps.rearrange("p (h d) -> p h d", h=NH))
            # softmax-denominator ones column (col 64 of each head)
            nc.vector.tensor_copy(vP[st][:, :, HD], ones_sb)
    wv_pool.release()

    def attention_block(c, qb, apool=None):
        apool = apool or psatt
        last_kt = 4 * qb + 3
        att_ps = [apool.tile([HD + 1, 512], F32, name="att_ps",
                             tag="att_ps") for _ in range(2)]
        for kt in range(last_kt + 1):
            if kt < 4 * qb:
                cs, diag = 0, False
            else:
                d0 = 128 * kt - 512 * qb
                cs, diag = min(d0, 256), True
            w = 512 - cs
            for u in range(2):
                po = u * HD
                h = 2 * c + u
                s_ps = pst.tile([128, 512], F32, name="s_ps", tag="s_ps")
                nc.tensor.matmul(
                    s_ps[:, cs:512],
                    lhsT=kT_sb[c][po:po + HD, kt * 128:(kt + 1) * 128],
                    rhs=qT_sb[c][po:po + HD, qb * 512 + cs:(qb + 1) * 512],
                    start=True, stop=True, skip_group_check=True,
                    tile_position=(po, 0))
                pt = ptp.tile([128, 512], F32R, name="pt", tag="pt")
                nc.scalar.activation(
                    pt[:, cs:512], s_ps[:, cs:512], EXP, scale=0.125)
                if diag:
                    # zero invalid (k > q):
                    # valid iff (512*qb + cs + y) - (128*kt + x) >= 0
                    nc.gpsimd.affine_select(
                        out=pt[:, cs:512], in_=pt[:, cs:512],
                        compare_op=GE, fill=0.0,
                        base=512 * qb + cs - 128 * kt,
                        channel_multiplier=-1,
                        pattern=[[1, w]])
                nc.tensor.matmul(
                    att_ps[u][:, cs:512],
                    lhsT=vP[kt][:, h, :],
                    rhs=pt[:, cs:512],
                    start=(kt == 0), stop=(kt == last_kt),
                    skip_group_check=True)
        for u in range(2):
            po = u * HD
            l_sb = smalls.tile([1, 512], F32, name="l_sb", tag="l_sb")
            nc.vector.tensor_copy(l_sb, att_ps[u][HD:HD + 1, :])
            r_sb = smalls.tile([1, 512], F32, name="r_sb", tag="r_sb")
            nc.vector.reciprocal_approx_fast(out=r_sb, in_=l_sb)
            rb_sb = smalls.tile([HD, 512], F32, name="rb_sb", tag="rb_sb")
            nc.gpsimd.partition_broadcast(rb_sb, r_sb)
            nc.vector.tensor_mul(
                att_n[c][po:po + HD, qb * 512:(qb + 1) * 512],
                att_ps[u][0:HD, :], rb_sb)

    def outproj(qb):
        for s4 in range(4):
            st = qb * 4 + s4
            for eb in range(2):
                ps = po_pool.tile([128, 512], F32, name="ps_o", tag="ps_o")
                for c in range(4):
                    nc.tensor.matmul(
                        ps,
                        lhsT=att_n[c][:, st * 128:(st + 1) * 128],
                        rhs=wo_sb[c][:, eb * 512:(eb + 1) * 512],
                        start=(c == 0), stop=(c == 3), skip_group_check=True)
                o_sb = ostage.tile([128, 512], F32, name="o_sb", tag="o_sb")
                nc.vector.tensor_copy(o_sb, ps)
                nc.sync.dma_start(
                    out=out[st * 128:(st + 1) * 128,
                            eb * 512:(eb + 1) * 512],
                    in_=o_sb)

    # ---- wavefront: Q/K projections for pairs 1..3 interleaved with ----
    # ---- attention blocks of already-projected pairs                ----
    for c in range(1, 4):
        wt = load_wqk(c)
        for sb_i in range(SB):
            xtc = load_xtc(sb_i)
            qk_proj(c, sb_i, xtc, wt)
        # attention anti-diagonal: blocks with pair + qb budget available
        for cc in range(c):
            qb = c - 1 - cc
            attention_block(cc, qb)
    wqk_pool.release()
    xpool.release()
    pp.release()

    po_pool = tc.alloc_tile_pool(name="po", bufs=2, space="PSUM")
    wopool = tc.alloc_tile_pool(name="wopool", bufs=1)
    ostage = tc.alloc_tile_pool(name="ostage", bufs=2)
    wo_sb = []
    for c in range(4):
        t = wopool.tile([128, E], F32R, name=f"wo{c}", tag=f"wo{c}")
        nc.sync.dma_start(out=t, in_=wo[c * 128:(c + 1) * 128, :])
        wo_sb.append(t)

    # remaining anti-diagonals; outproj(qb) as soon as all pairs reach qb
    for d in range(3, 7):
        blocks = [(cc, d - cc) for cc in range(4) if 0 <= d - cc <= 3]
        for i, (cc, qb) in enumerate(blocks):
            attention_block(cc, qb)
            if i == len(blocks) - 1:
                outproj(d - 3)

    # release in LIFO order per memory space
    ostage.release()
    wopool.release()
    smalls.release()
    ptp.release()
    po_pool.release()
    psatt.release()
    pst.release()
    persist1.release()


def _build():
    if "nc" in _CACHE:
        return _CACHE["nc"]
    nc = bacc.Bacc()
    with tile.TileContext(nc) as tc:
        _emit(nc, tc)
    nc.compile()
    _CACHE["nc"] = nc
    return nc


def _make_in_maps(inputs):
    x = np.asarray(inputs["x"], dtype=np.float32)
    Wq = np.asarray(inputs["Wq"], dtype=np.float32)
    Wk = np.asarray(inputs["Wk"], dtype=np.float32)
    Wv = np.asarray(inputs["Wv"], dtype=np.float32)
    Wo = np.asarray(inputs["Wo"], dtype=np.float32)
    in_maps = []
    for core in range(8):
        b, g = core // 2, core % 2
        cols = slice(g * G, (g + 1) * G)
        in_maps.append({
            "xT": np.ascontiguousarray(x[b].T),
            "wq": np.ascontiguousarray(Wq[:, cols]),
            "wk": np.ascontiguousarray(Wk[:, cols]),
            "wv": np.ascontiguousarray(Wv[:, cols]),
            "wo": np.ascontiguousarray(Wo[cols, :]),
            "c_ones": np.ones((128, 128), dtype=np.float32),
        })
    return in_maps


def kernel(x, Wq, Wk, Wv, Wo):
    nc = _build()
    in_maps = _make_in_maps(dict(x=x, Wq=Wq, Wk=Wk, Wv=Wv, Wo=Wo))
    res = run_bass_kernel_spmd(nc, in_maps, core_ids=list(range(8)))
    out = np.zeros((4, S, E), dtype=np.float32)
    for core in range(8):
        out[core // 2] += res.results[core]["out"]
    return out


if __name__ == "__main__":
    rng = np.random.default_rng(0)
    x = rng.standard_normal((4, S, E), dtype=np.float32)
    sc = 1.0 / np.sqrt(E)
    Wq = rng.standard_normal((E, E), dtype=np.float32) * sc
    Wk = rng.standard_normal((E, E), dtype=np.float32) * sc
    Wv = rng.standard_normal((E, E), dtype=np.float32) * sc
    Wo = rng.standard_normal((E, E), dtype=np.float32) * sc
    o = kernel(x, Wq, Wk, Wv, Wo)
    print("out", o.shape, o.dtype, np.abs(o).mean())
